# revision 1
# baseline (speedup 1.0000x reference)
"""Trainium2 Bass kernel for nn_Damping_layer: out = kipf_term - lbda[:, None] * input_term.

Sharding: pure row-parallel over the n_nodes axis across 8 NeuronCores
(12500 rows per core), no cross-core communication.

Per-core layout: rows are tiled into [128 partitions x 8 rows/partition]
tiles (1024 rows = 1 MiB per f32 tile), so each partition's DMA run is
8 KiB contiguous. lbda is pre-shuffled on host into the matching
[partition, group] layout so the fused DVE op
    out = (input * (-lbda)) + kipf            (InstTensorScalarPtr)
can consume it as a per-partition scalar, one op per 128-row group.
"""

import numpy as np

N_NODES = 100000
N_FEAT = 256
N_CORES = 8
ROWS_PER_CORE = N_NODES // N_CORES  # 12500

R_PP = 8                       # rows per partition in a full tile
TILE_ROWS = 128 * R_PP         # 1024 rows per tile
N_FULL = ROWS_PER_CORE // TILE_ROWS            # 12 full tiles
MAIN_ROWS = N_FULL * TILE_ROWS                 # 12288
REM_ROWS = ROWS_PER_CORE - MAIN_ROWS           # 212
REM_P, REM_J = 53, 4                           # 53 partitions x 4 rows
LB_COLS = N_FULL * R_PP + REM_J                # 100

_CACHE = {}


def _build_nc():
    from contextlib import ExitStack

    import concourse.bacc as bacc
    import concourse.mybir as mybir
    import concourse.tile as tile

    FP32 = mybir.dt.float32
    nc = bacc.Bacc(
        "TRN2", target_bir_lowering=False, debug=False, num_devices=N_CORES
    )
    x = nc.dram_tensor("x", [ROWS_PER_CORE, N_FEAT], FP32, kind="ExternalInput").ap()
    k = nc.dram_tensor("k", [ROWS_PER_CORE, N_FEAT], FP32, kind="ExternalInput").ap()
    lb = nc.dram_tensor("lb", [128, LB_COLS], FP32, kind="ExternalInput").ap()
    o = nc.dram_tensor("o", [ROWS_PER_CORE, N_FEAT], FP32, kind="ExternalOutput").ap()

    # [t, p, (j c)] views: partition p holds rows tile_base + p*R_PP + j,
    # i.e. R_PP*1KB contiguous DRAM per partition.
    xv = x[0:MAIN_ROWS].rearrange("(t p j) c -> t p (j c)", t=N_FULL, p=128, j=R_PP)
    kv = k[0:MAIN_ROWS].rearrange("(t p j) c -> t p (j c)", t=N_FULL, p=128, j=R_PP)
    ov = o[0:MAIN_ROWS].rearrange("(t p j) c -> t p (j c)", t=N_FULL, p=128, j=R_PP)
    xr = x[MAIN_ROWS:ROWS_PER_CORE].rearrange("(p j) c -> p (j c)", p=REM_P, j=REM_J)
    kr = k[MAIN_ROWS:ROWS_PER_CORE].rearrange("(p j) c -> p (j c)", p=REM_P, j=REM_J)
    orr = o[MAIN_ROWS:ROWS_PER_CORE].rearrange("(p j) c -> p (j c)", p=REM_P, j=REM_J)

    MULT = mybir.AluOpType.mult
    ADD = mybir.AluOpType.add

    with tile.TileContext(nc) as tc, ExitStack() as ctx:
        const = ctx.enter_context(tc.tile_pool(name="const", bufs=1))
        xpool = ctx.enter_context(tc.tile_pool(name="xp", bufs=3))
        kpool = ctx.enter_context(tc.tile_pool(name="kp", bufs=3))
        opool = ctx.enter_context(tc.tile_pool(name="op", bufs=3))

        lbt = const.tile([128, LB_COLS], FP32)
        nc.sync.dma_start(out=lbt[:], in_=lb[:])
        nlb = const.tile([128, LB_COLS], FP32)
        nc.vector.tensor_scalar_mul(nlb[:], lbt[:], -1.0)

        def emit_tile(xin, kin, oout, parts, jj, lb_col0):
            fd = jj * N_FEAT
            xt = xpool.tile([128, R_PP * N_FEAT], FP32, tag="xt")
            nc.sync.dma_start(out=xt[:parts, :fd], in_=xin)
            kt = kpool.tile([128, R_PP * N_FEAT], FP32, tag="kt")
            nc.sync.dma_start(out=kt[:parts, :fd], in_=kin)
            ot = opool.tile([128, R_PP * N_FEAT], FP32, tag="ot")
            for j in range(jj):
                s = slice(j * N_FEAT, (j + 1) * N_FEAT)
                nc.vector.scalar_tensor_tensor(
                    out=ot[:parts, s],
                    in0=xt[:parts, s],
                    scalar=nlb[:parts, lb_col0 + j : lb_col0 + j + 1],
                    in1=kt[:parts, s],
                    op0=MULT,
                    op1=ADD,
                )
            nc.scalar.dma_start(out=oout, in_=ot[:parts, :fd])

        for t in range(N_FULL):
            emit_tile(xv[t], kv[t], ov[t], 128, R_PP, t * R_PP)
        emit_tile(xr, kr, orr, REM_P, REM_J, N_FULL * R_PP)

    nc.compile()
    return nc


def _get_nc():
    if "nc" not in _CACHE:
        _CACHE["nc"] = _build_nc()
    return _CACHE["nc"]


def _shuffle_lbda(lb_core):
    """[12500] -> [128, LB_COLS] so nlb[p, t*R_PP+j] = lbda[t*1024 + p*8 + j]."""
    out = np.zeros((128, LB_COLS), np.float32)
    main = lb_core[:MAIN_ROWS].reshape(N_FULL, 128, R_PP)
    out[:, : N_FULL * R_PP] = main.transpose(1, 0, 2).reshape(128, N_FULL * R_PP)
    out[:REM_P, N_FULL * R_PP :] = lb_core[MAIN_ROWS:].reshape(REM_P, REM_J)
    return out


def _make_in_maps(input_term, kipf_term, lbda):
    input_term = np.ascontiguousarray(np.asarray(input_term, dtype=np.float32))
    kipf_term = np.ascontiguousarray(np.asarray(kipf_term, dtype=np.float32))
    lbda = np.asarray(lbda, dtype=np.float32)
    in_maps = []
    for c in range(N_CORES):
        sl = slice(c * ROWS_PER_CORE, (c + 1) * ROWS_PER_CORE)
        in_maps.append(
            {
                "x": input_term[sl],
                "k": kipf_term[sl],
                "lb": _shuffle_lbda(lbda[sl]),
            }
        )
    return in_maps


def kernel(input_term, kipf_term, lbda, spar=None, **_unused):
    from concourse.bass_utils import run_bass_kernel_spmd

    nc = _get_nc()
    in_maps = _make_in_maps(input_term, kipf_term, lbda)
    res = run_bass_kernel_spmd(nc, in_maps, list(range(N_CORES))).results
    return np.concatenate([res[c]["o"] for c in range(N_CORES)], axis=0)


# revision 3
# speedup vs baseline: 1.1151x; 1.1151x over previous
"""Trainium2 Bass kernel for nn_Damping_layer: out = kipf_term - lbda[:, None] * input_term.

Sharding: pure row-parallel over the n_nodes axis across 8 NeuronCores
(12500 rows per core), no cross-core communication. Each core's shard is
host-padded to 12544 rows so it divides into 14 uniform tiles of
[128 partitions x 7 rows/partition] (896 KiB f32), giving every DMA
7 KiB-contiguous runs per partition across all 16 SDMA engines.

lbda is pre-shuffled on host into the matching [partition, group] layout
so the fused DVE op
    out = (input * (-lbda)) + kipf            (InstTensorScalarPtr)
consumes it as a per-partition scalar, one op per 128-row group.
"""

import numpy as np

N_NODES = 100000
N_FEAT = 256
N_CORES = 8
ROWS_PER_CORE = N_NODES // N_CORES  # 12500

R_PP = 7                        # rows per partition in a tile
TILE_ROWS = 128 * R_PP          # 896 rows per tile
N_TILES = 14                    # tiles per core
PAD_ROWS = N_TILES * TILE_ROWS  # 12544 rows per core after padding
LB_COLS = N_TILES * R_PP        # 98
N_BUFS = 5

_CACHE = {}


def _build_nc():
    from contextlib import ExitStack

    import concourse.bacc as bacc
    import concourse.mybir as mybir
    import concourse.tile as tile

    FP32 = mybir.dt.float32
    nc = bacc.Bacc(
        "TRN2", target_bir_lowering=False, debug=False, num_devices=N_CORES
    )
    x = nc.dram_tensor("x", [PAD_ROWS, N_FEAT], FP32, kind="ExternalInput").ap()
    k = nc.dram_tensor("k", [PAD_ROWS, N_FEAT], FP32, kind="ExternalInput").ap()
    lb = nc.dram_tensor("lb", [128, LB_COLS], FP32, kind="ExternalInput").ap()
    o = nc.dram_tensor("o", [PAD_ROWS, N_FEAT], FP32, kind="ExternalOutput").ap()

    # [t, p, (j c)] views: partition p holds rows tile_base + p*R_PP + j,
    # i.e. R_PP*1KB contiguous DRAM per partition.
    xv = x.rearrange("(t p j) c -> t p (j c)", t=N_TILES, p=128, j=R_PP)
    kv = k.rearrange("(t p j) c -> t p (j c)", t=N_TILES, p=128, j=R_PP)
    ov = o.rearrange("(t p j) c -> t p (j c)", t=N_TILES, p=128, j=R_PP)

    MULT = mybir.AluOpType.mult
    ADD = mybir.AluOpType.add

    with tile.TileContext(nc) as tc, ExitStack() as ctx:
        const = ctx.enter_context(tc.tile_pool(name="const", bufs=1))
        xpool = ctx.enter_context(tc.tile_pool(name="xp", bufs=N_BUFS))
        kpool = ctx.enter_context(tc.tile_pool(name="kp", bufs=N_BUFS))
        opool = ctx.enter_context(tc.tile_pool(name="op", bufs=N_BUFS))

        lbt = const.tile([128, LB_COLS], FP32)
        nc.sync.dma_start(out=lbt[:], in_=lb[:])
        nlb = const.tile([128, LB_COLS], FP32)
        nc.vector.tensor_scalar_mul(nlb[:], lbt[:], -1.0)

        for t in range(N_TILES):
            xt = xpool.tile([128, R_PP * N_FEAT], FP32, tag="xt")
            nc.sync.dma_start(out=xt[:], in_=xv[t])
            kt = kpool.tile([128, R_PP * N_FEAT], FP32, tag="kt")
            nc.sync.dma_start(out=kt[:], in_=kv[t])
            ot = opool.tile([128, R_PP * N_FEAT], FP32, tag="ot")
            for j in range(R_PP):
                s = slice(j * N_FEAT, (j + 1) * N_FEAT)
                c = t * R_PP + j
                nc.vector.scalar_tensor_tensor(
                    out=ot[:, s],
                    in0=xt[:, s],
                    scalar=nlb[:, c : c + 1],
                    in1=kt[:, s],
                    op0=MULT,
                    op1=ADD,
                )
            nc.scalar.dma_start(out=ov[t], in_=ot[:])

    nc.compile()
    return nc


def _get_nc():
    if "nc" not in _CACHE:
        _CACHE["nc"] = _build_nc()
    return _CACHE["nc"]


def _shuffle_lbda(lb_core):
    """[PAD_ROWS] -> [128, LB_COLS] with lb[p, t*R_PP+j] = lbda[t*896 + p*7 + j]."""
    return np.ascontiguousarray(
        lb_core.reshape(N_TILES, 128, R_PP)
        .transpose(1, 0, 2)
        .reshape(128, LB_COLS)
    )


def _make_in_maps(input_term, kipf_term, lbda):
    input_term = np.asarray(input_term, dtype=np.float32)
    kipf_term = np.asarray(kipf_term, dtype=np.float32)
    lbda = np.asarray(lbda, dtype=np.float32)
    pad = N_CORES * PAD_ROWS - N_NODES  # total pad rows if done globally
    in_maps = []
    for c in range(N_CORES):
        sl = slice(c * ROWS_PER_CORE, (c + 1) * ROWS_PER_CORE)
        xpadded = np.zeros((PAD_ROWS, N_FEAT), np.float32)
        xpadded[:ROWS_PER_CORE] = input_term[sl]
        kpadded = np.zeros((PAD_ROWS, N_FEAT), np.float32)
        kpadded[:ROWS_PER_CORE] = kipf_term[sl]
        lpadded = np.zeros((PAD_ROWS,), np.float32)
        lpadded[:ROWS_PER_CORE] = lbda[sl]
        in_maps.append(
            {"x": xpadded, "k": kpadded, "lb": _shuffle_lbda(lpadded)}
        )
    return in_maps


def kernel(input_term, kipf_term, lbda, spar=None, **_unused):
    from concourse.bass_utils import run_bass_kernel_spmd

    nc = _get_nc()
    in_maps = _make_in_maps(input_term, kipf_term, lbda)
    res = run_bass_kernel_spmd(nc, in_maps, list(range(N_CORES))).results
    return np.concatenate(
        [res[c]["o"][:ROWS_PER_CORE] for c in range(N_CORES)], axis=0
    )


# revision 4
# speedup vs baseline: 1.2357x; 1.1081x over previous
"""Trainium2 Bass kernel for nn_Damping_layer: out = kipf_term - lbda[:, None] * input_term.

Sharding: pure row-parallel over the n_nodes axis across 8 NeuronCores
(12500 rows per core), no cross-core communication. Each core's shard is
host-padded to 12544 rows so it divides into 14 uniform tiles of
[128 partitions x 7 rows/partition] (896 KiB f32), giving every DMA
7 KiB-contiguous runs per partition across all 16 SDMA engines.

lbda is pre-shuffled on host into the matching [partition, group] layout
so the fused DVE op
    out = (input * (-lbda)) + kipf            (InstTensorScalarPtr)
consumes it as a per-partition scalar, one op per 128-row group.
"""

import numpy as np

N_NODES = 100000
N_FEAT = 256
N_CORES = 8
ROWS_PER_CORE = N_NODES // N_CORES  # 12500

R_PP = 7                        # rows per partition in a tile
TILE_ROWS = 128 * R_PP          # 896 rows per tile
N_TILES = 14                    # tiles per core
PAD_ROWS = N_TILES * TILE_ROWS  # 12544 rows per core after padding
LB_COLS = N_TILES * R_PP        # 98
N_BUFS = 5

_CACHE = {}


def _build_nc():
    from contextlib import ExitStack

    import concourse.bacc as bacc
    import concourse.mybir as mybir
    import concourse.tile as tile

    FP32 = mybir.dt.float32
    nc = bacc.Bacc(
        "TRN2", target_bir_lowering=False, debug=False, num_devices=N_CORES
    )
    x = nc.dram_tensor("x", [PAD_ROWS, N_FEAT], FP32, kind="ExternalInput").ap()
    k = nc.dram_tensor("k", [PAD_ROWS, N_FEAT], FP32, kind="ExternalInput").ap()
    lb = nc.dram_tensor("lb", [128, LB_COLS], FP32, kind="ExternalInput").ap()
    o = nc.dram_tensor("o", [PAD_ROWS, N_FEAT], FP32, kind="ExternalOutput").ap()

    # [t, p, (j c)] views: partition p holds rows tile_base + p*R_PP + j,
    # i.e. R_PP*1KB contiguous DRAM per partition.
    xv = x.rearrange("(t p j) c -> t p (j c)", t=N_TILES, p=128, j=R_PP)
    kv = k.rearrange("(t p j) c -> t p (j c)", t=N_TILES, p=128, j=R_PP)
    ov = o.rearrange("(t p j) c -> t p (j c)", t=N_TILES, p=128, j=R_PP)

    MULT = mybir.AluOpType.mult
    ADD = mybir.AluOpType.add

    with tile.TileContext(nc) as tc, ExitStack() as ctx:
        const = ctx.enter_context(tc.tile_pool(name="const", bufs=1))
        xpool = ctx.enter_context(tc.tile_pool(name="xp", bufs=N_BUFS))
        kpool = ctx.enter_context(tc.tile_pool(name="kp", bufs=N_BUFS))
        opool = ctx.enter_context(tc.tile_pool(name="op", bufs=N_BUFS))

        lbt = const.tile([128, LB_COLS], FP32)
        nc.sync.dma_start(out=lbt[:], in_=lb[:])
        nlb = const.tile([128, LB_COLS], FP32)
        nc.vector.tensor_scalar_mul(nlb[:], lbt[:], -1.0)

        for t in range(N_TILES):
            xt = xpool.tile([128, R_PP * N_FEAT], FP32, tag="xt")
            nc.sync.dma_start(out=xt[:], in_=xv[t])
            kt = kpool.tile([128, R_PP * N_FEAT], FP32, tag="kt")
            nc.gpsimd.dma_start(out=kt[:], in_=kv[t])
            ot = opool.tile([128, R_PP * N_FEAT], FP32, tag="ot")
            for j in range(R_PP):
                s = slice(j * N_FEAT, (j + 1) * N_FEAT)
                c = t * R_PP + j
                nc.vector.scalar_tensor_tensor(
                    out=ot[:, s],
                    in0=xt[:, s],
                    scalar=nlb[:, c : c + 1],
                    in1=kt[:, s],
                    op0=MULT,
                    op1=ADD,
                )
            nc.scalar.dma_start(out=ov[t], in_=ot[:])

    nc.compile()
    return nc


def _get_nc():
    if "nc" not in _CACHE:
        _CACHE["nc"] = _build_nc()
    return _CACHE["nc"]


def _shuffle_lbda(lb_core):
    """[PAD_ROWS] -> [128, LB_COLS] with lb[p, t*R_PP+j] = lbda[t*896 + p*7 + j]."""
    return np.ascontiguousarray(
        lb_core.reshape(N_TILES, 128, R_PP)
        .transpose(1, 0, 2)
        .reshape(128, LB_COLS)
    )


def _make_in_maps(input_term, kipf_term, lbda):
    input_term = np.asarray(input_term, dtype=np.float32)
    kipf_term = np.asarray(kipf_term, dtype=np.float32)
    lbda = np.asarray(lbda, dtype=np.float32)
    pad = N_CORES * PAD_ROWS - N_NODES  # total pad rows if done globally
    in_maps = []
    for c in range(N_CORES):
        sl = slice(c * ROWS_PER_CORE, (c + 1) * ROWS_PER_CORE)
        xpadded = np.zeros((PAD_ROWS, N_FEAT), np.float32)
        xpadded[:ROWS_PER_CORE] = input_term[sl]
        kpadded = np.zeros((PAD_ROWS, N_FEAT), np.float32)
        kpadded[:ROWS_PER_CORE] = kipf_term[sl]
        lpadded = np.zeros((PAD_ROWS,), np.float32)
        lpadded[:ROWS_PER_CORE] = lbda[sl]
        in_maps.append(
            {"x": xpadded, "k": kpadded, "lb": _shuffle_lbda(lpadded)}
        )
    return in_maps


def kernel(input_term, kipf_term, lbda, spar=None, **_unused):
    from concourse.bass_utils import run_bass_kernel_spmd

    nc = _get_nc()
    in_maps = _make_in_maps(input_term, kipf_term, lbda)
    res = run_bass_kernel_spmd(nc, in_maps, list(range(N_CORES))).results
    return np.concatenate(
        [res[c]["o"][:ROWS_PER_CORE] for c in range(N_CORES)], axis=0
    )
